# revision 2
# baseline (speedup 1.0000x reference)
# Bidirectional cross-attention Trainium2 kernel (Bass/Tile), 8-core head-parallel.
#
# Sharding: 16 heads / 8 cores = 2 heads per core (tensor-parallel on h).
# Each core computes its 2 heads' QK/V projections, the 2048x2048 similarity,
# exp (no max-subtraction: inputs are small-scale so exp is safe in fp32),
# both softmax directions via row/col sums obtained from a ones-column in the
# V operands, both attention outputs, and its partial (row-parallel) slice of
# the final projections. Host sums the 8 partials and adds biases (unshard).
#
# Math per head (E = exp(scale*sim), computed tile-by-tile, never stored):
#   pass 1: H^T[d,j]   = sum_i v[i,d]*E[i,j]   (+ colsum row via ones col)
#   pass 2: G^T[d,i]   = sum_j cv[j,d]*E[i,j]  (+ rowsum row via ones col)
#   out_h^T  = G^T / rowsum ; ctxout_h^T = H^T / colsum (PE ones-broadcast + DVE mul)
# Pass 2 recomputes sim transposed (cheaper on PE than transposing E).

import os
import sys

for _p in ("/opt/trn_rl_repo", "/root/.axon_site/_ro/trn_rl_repo"):
    if os.path.isdir(_p) and _p not in sys.path:
        sys.path.insert(0, _p)

import numpy as np
import ml_dtypes

HEADS = 16
DIM_HEAD = 64
DIM = 1024
SEQ = 2048
N_CORES = 8
HPC = HEADS // N_CORES          # heads per core = 2
FPC = HPC * DIM_HEAD            # feature cols per core = 128
SCALE = DIM_HEAD ** -0.5


def _ts(i, size):
    return slice(i * size, (i + 1) * size)


def build_bass(seq=SEQ, dim=DIM, fpc=FPC, hpc=HPC, num_devices=N_CORES):
    """Build and compile the per-core Bass program (SPMD: same NEFF on all cores)."""
    import concourse.bacc as bacc
    import concourse.tile as tile
    import concourse.mybir as mybir

    f32 = mybir.dt.float32
    bf16 = mybir.dt.bfloat16
    f16 = mybir.dt.float16
    Exp = mybir.ActivationFunctionType.Exp

    P = 128
    KT = dim // P              # k tiles of contraction over DIM (8)
    NT = seq // P              # 128-blocks along sequence (16)
    NCH = seq // 512           # 512-chunks along sequence (4)
    OCH = dim // 512           # 512-chunks of output dim (2)
    dh = DIM_HEAD

    nc = bacc.Bacc("TRN2", target_bir_lowering=False, debug=False,
                   num_devices=num_devices)

    xT = nc.dram_tensor("xT", (dim, seq), bf16, kind="ExternalInput").ap()
    cT = nc.dram_tensor("cT", (dim, seq), bf16, kind="ExternalInput").ap()
    wqk = nc.dram_tensor("wqk", (dim, fpc), bf16, kind="ExternalInput").ap()
    wv = nc.dram_tensor("wv", (dim, fpc), bf16, kind="ExternalInput").ap()
    wcqk = nc.dram_tensor("wcqk", (dim, fpc), bf16, kind="ExternalInput").ap()
    wcv = nc.dram_tensor("wcv", (dim, fpc), bf16, kind="ExternalInput").ap()
    wout = nc.dram_tensor("wout", (fpc, dim), bf16, kind="ExternalInput").ap()
    wcout = nc.dram_tensor("wcout", (fpc, dim), bf16, kind="ExternalInput").ap()
    out_p = nc.dram_tensor("out_p", (seq, dim), f16, kind="ExternalOutput").ap()
    ctx_p = nc.dram_tensor("ctx_p", (seq, dim), f16, kind="ExternalOutput").ap()

    with tile.TileContext(nc) as tc:
        from contextlib import ExitStack
        with ExitStack() as ctx:
            persist = ctx.enter_context(tc.tile_pool(name="persist", bufs=1))
            psum512 = ctx.enter_context(
                tc.tile_pool(name="ps512", bufs=3, space="PSUM"))
            psum_acc = ctx.enter_context(
                tc.tile_pool(name="psacc", bufs=1, space="PSUM"))
            e_pool = ctx.enter_context(tc.tile_pool(name="epool", bufs=4))
            ht_pool = ctx.enter_context(tc.tile_pool(name="htpool", bufs=2))
            norm_pool = ctx.enter_context(tc.tile_pool(name="normpool", bufs=2))
            fin_pool = ctx.enter_context(tc.tile_pool(name="finpool", bufs=4))

            # ---- load inputs to SBUF ----
            xT_sb = persist.tile([P, KT, seq], bf16, tag="xT")
            nc.sync.dma_start(xT_sb, xT.rearrange("(kt p) i -> p kt i", p=P))
            cT_sb = persist.tile([P, KT, seq], bf16, tag="cT")
            nc.sync.dma_start(cT_sb, cT.rearrange("(kt p) i -> p kt i", p=P))

            w_sbs = {}
            for name, ap_ in (("wqk", wqk), ("wv", wv), ("wcqk", wcqk),
                              ("wcv", wcv)):
                t = persist.tile([P, KT, fpc], bf16, tag=name)
                nc.sync.dma_start(t, ap_.rearrange("(kt p) f -> p kt f", p=P))
                w_sbs[name] = t
            wout_sb = persist.tile([P, dim], bf16, tag="wout")
            nc.sync.dma_start(wout_sb, wout)
            wcout_sb = persist.tile([P, dim], bf16, tag="wcout")
            nc.sync.dma_start(wcout_sb, wcout)

            ones_sb = persist.tile([1, dh], f32, tag="ones")
            nc.vector.memset(ones_sb, 1.0)

            # ---- projections ----
            # qkT/cqkT: [P(f), seq] transposed layout (f = hpc*dh head dims)
            qkT_sb = persist.tile([P, seq], bf16, tag="qkT")
            cqkT_sb = persist.tile([P, seq], bf16, tag="cqkT")
            for src_sb, wname, dst in ((xT_sb, "wqk", qkT_sb),
                                       (cT_sb, "wcqk", cqkT_sb)):
                w_sb = w_sbs[wname]
                for icx in range(NCH):
                    ps = psum512.tile([P, 512], f32, tag="ps512")
                    for kt in range(KT):
                        nc.tensor.matmul(ps, w_sb[:, kt], src_sb[:, kt, _ts(icx, 512)],
                                         start=(kt == 0), stop=(kt == KT - 1))
                    nc.vector.tensor_copy(dst[:, _ts(icx, 512)], ps)

            # v/cv: natural [P(i), NT, hpc*(dh+1)] with a ones column per head
            vw = dh + 1
            v_sb = persist.tile([P, NT, hpc * vw], bf16, tag="v")
            cv_sb = persist.tile([P, NT, hpc * vw], bf16, tag="cv")
            for h in range(hpc):
                nc.vector.memset(v_sb[:, :, h * vw + dh], 1.0)
                nc.vector.memset(cv_sb[:, :, h * vw + dh], 1.0)
            for src_sb, wname, dst in ((xT_sb, "wv", v_sb), (cT_sb, "wcv", cv_sb)):
                w_sb = w_sbs[wname]
                for ib in range(NT):
                    ps = psum512.tile([P, P], f32, tag="ps512")
                    for kt in range(KT):
                        nc.tensor.matmul(ps, src_sb[:, kt, _ts(ib, P)], w_sb[:, kt],
                                         start=(kt == 0), stop=(kt == KT - 1))
                    for h in range(hpc):
                        nc.vector.tensor_copy(dst[:, ib, h * vw:h * vw + dh],
                                              ps[:, _ts(h, dh)])

            # ---- per-head attention ----
            outmT_sb = persist.tile([P, seq], bf16, tag="outmT")
            ctxmT_sb = persist.tile([P, seq], bf16, tag="ctxmT")

            for h in range(hpc):
                hs = slice(h * dh, (h + 1) * dh)
                va = slice(h * vw, h * vw + vw)

                # pass 1: H^T (+colsum) accumulated over i tiles
                psH = psum_acc.tile([vw, seq], f32, tag="acc")
                for jc in range(NCH):
                    for it in range(NT):
                        ps = psum512.tile([P, 512], f32, tag="ps512")
                        nc.tensor.matmul(ps, qkT_sb[hs, _ts(it, P)],
                                         cqkT_sb[hs, _ts(jc, 512)],
                                         start=True, stop=True)
                        e = e_pool.tile([P, 512], bf16, tag="e")
                        nc.scalar.activation(e, ps, Exp, scale=SCALE)
                        nc.tensor.matmul(psH[:, _ts(jc, 512)], v_sb[:, it, va], e,
                                         start=(it == 0), stop=(it == NT - 1))
                hT = ht_pool.tile([vw, seq], f32, tag="ht")
                nc.vector.tensor_copy(hT, psH)

                # pass 2: G^T (+rowsum) accumulated over j tiles (sim recomputed
                # transposed)
                psG = psum_acc.tile([vw, seq], f32, tag="acc")
                for icx in range(NCH):
                    for jt in range(NT):
                        ps = psum512.tile([P, 512], f32, tag="ps512")
                        nc.tensor.matmul(ps, cqkT_sb[hs, _ts(jt, P)],
                                         qkT_sb[hs, _ts(icx, 512)],
                                         start=True, stop=True)
                        et = e_pool.tile([P, 512], bf16, tag="e")
                        nc.scalar.activation(et, ps, Exp, scale=SCALE)
                        nc.tensor.matmul(psG[:, _ts(icx, 512)], cv_sb[:, jt, va], et,
                                         start=(jt == 0), stop=(jt == NT - 1))

                gT = ht_pool.tile([vw, seq], f32, tag="gt")
                nc.vector.tensor_copy(gT, psG)

                # normalize: out^T = G^T * (1/rowsum), ctxout^T = H^T * (1/colsum)
                rrs = norm_pool.tile([1, seq], f32, tag="rrs")
                nc.vector.reciprocal(rrs, psG[dh:dh + 1, :])
                rcs = norm_pool.tile([1, seq], f32, tag="rcs")
                nc.vector.reciprocal(rcs, hT[dh:dh + 1, :])
                for chv in range(NCH):
                    bc = psum512.tile([dh, 512], f32, tag="ps512")
                    nc.tensor.matmul(bc, ones_sb, rrs[:, _ts(chv, 512)],
                                     start=True, stop=True)
                    nc.vector.tensor_mul(outmT_sb[hs, _ts(chv, 512)],
                                         gT[0:dh, _ts(chv, 512)], bc)
                    bc2 = psum512.tile([dh, 512], f32, tag="ps512")
                    nc.tensor.matmul(bc2, ones_sb, rcs[:, _ts(chv, 512)],
                                     start=True, stop=True)
                    nc.vector.tensor_mul(ctxmT_sb[hs, _ts(chv, 512)],
                                         hT[0:dh, _ts(chv, 512)], bc2)

            # ---- final row-parallel projections (partial sums, fp16 out) ----
            out_view = out_p.rearrange("(ib p) o -> p ib o", p=P)
            ctx_view = ctx_p.rearrange("(ib p) o -> p ib o", p=P)
            for si, (mT, w_sb, odram) in enumerate(
                    ((outmT_sb, wout_sb, out_view), (ctxmT_sb, wcout_sb, ctx_view))):
                for ib in range(NT):
                    for oc in range(OCH):
                        pso = psum512.tile([P, 512], f32, tag="ps512")
                        nc.tensor.matmul(pso, mT[:, _ts(ib, P)], w_sb[:, _ts(oc, 512)],
                                         start=True, stop=True)
                        osb = fin_pool.tile([P, 512], f16, tag="osb")
                        if si == 0:
                            nc.scalar.copy(osb, pso)
                        else:
                            nc.vector.tensor_copy(osb, pso)
                        nc.sync.dma_start(odram[:, ib, _ts(oc, 512)], osb)

    nc.compile()
    return nc


_NC_CACHE = {}


def _get_nc():
    if "nc" not in _NC_CACHE:
        _NC_CACHE["nc"] = build_bass()
    return _NC_CACHE["nc"]


def make_in_maps(x, context, W_qk, W_cqk, W_v, W_cv):
    bf = ml_dtypes.bfloat16
    xT = np.ascontiguousarray(x[0].T).astype(bf)
    cT = np.ascontiguousarray(context[0].T).astype(bf)
    in_maps = []
    for c in range(N_CORES):
        cs = _ts(c, FPC)
        in_maps.append({
            "xT": xT,
            "cT": cT,
            "wqk": np.ascontiguousarray(W_qk[:, cs]).astype(bf),
            "wv": np.ascontiguousarray(W_v[:, cs]).astype(bf),
            "wcqk": np.ascontiguousarray(W_cqk[:, cs]).astype(bf),
            "wcv": np.ascontiguousarray(W_cv[:, cs]).astype(bf),
        })
    return in_maps


def add_weight_slices(in_maps, W_out, W_cout):
    bf = ml_dtypes.bfloat16
    for c in range(N_CORES):
        rs = _ts(c, FPC)
        in_maps[c]["wout"] = np.ascontiguousarray(W_out[rs, :]).astype(bf)
        in_maps[c]["wcout"] = np.ascontiguousarray(W_cout[rs, :]).astype(bf)
    return in_maps


def kernel(x, context, W_qk, W_cqk, W_v, W_cv, W_out, b_out, W_cout, b_cout):
    from concourse.bass_utils import run_bass_kernel_spmd

    nc = _get_nc()
    in_maps = make_in_maps(np.asarray(x, np.float32), np.asarray(context, np.float32),
                           np.asarray(W_qk, np.float32), np.asarray(W_cqk, np.float32),
                           np.asarray(W_v, np.float32), np.asarray(W_cv, np.float32))
    add_weight_slices(in_maps, np.asarray(W_out, np.float32),
                      np.asarray(W_cout, np.float32))

    res = run_bass_kernel_spmd(nc, in_maps, core_ids=list(range(N_CORES)))

    out = np.zeros((SEQ, DIM), np.float32)
    ctx_out = np.zeros((SEQ, DIM), np.float32)
    for r in res.results:
        out += r["out_p"].astype(np.float32)
        ctx_out += r["ctx_p"].astype(np.float32)
    out += np.asarray(b_out, np.float32)
    ctx_out += np.asarray(b_cout, np.float32)
    return (out[None], ctx_out[None])
